# revision 8
# baseline (speedup 1.0000x reference)
"""CIDER criterion (DisLoss + CompLoss) on 8 Trainium2 NeuronCores.

Single launch per core. Host does index-driven data prep (the exact
per-sample EMA prototype scan, vectorized over classes), normalization,
fp8 quantization and layout; the device does the two large matmuls
(comp logits [1024 x 1024 x 512] per core, dis logits [128 x 1024 x 512])
plus the exp row-sums, which is >99.9% of the FLOPs.

Key design points:
- Operands pre-scaled by 16, quantized to fp8 e4m3 (TRN float8e4), in
  MatmulPerfMode.DoubleRow layout: SBUF tile [p, kb, j, col] contracts
  row kb*256 + j*128 + p. 0.5 cycles/output element = 2x bf16 rate.
- protoT columns are rolled per core so its own 125 dis classes always
  sit at columns 0..124 (comp row-sums are permutation invariant), so
  one static program serves all 8 cores with no per-core proto input.
- Inputs split into first-needed halves across the three DMA queues
  (HWDGE descriptor generation is ~20ns/descriptor on the issuing
  engine, so two 64-descriptor transfers beat one 128-descriptor one).
- PE warm-up dummies on a vector-memset bf16 tile run during the DMA
  window so the real matmuls start at full clock.
- Device ships raw exp row-sums [16, 128] (PE-transposed, 16 big
  descriptors); host applies ln, the exact e^10 diagonal correction
  (from the same fp8 values the PE sees), the 24-pad-column correction,
  and the exact positive term sum_c <cs_c, p_c> in f64.
"""

import numpy as np

# ---- problem constants (hardcoded per the harness contract) ----
B, C, D = 8192, 1000, 512
NCORES = 8
CPC = C // NCORES  # 125 classes per core
BPC = B // NCORES  # 1024 batch rows per core
P = 128
CPAD = 1024  # padded class columns (24 zero-prototype pads)
NPAD = CPAD - C
MT = BPC // P  # 8 batch tiles per core
KB = 2  # two (128 x 2)-row DoubleRow contraction blocks
NH = 512  # class-column half
SCALE = 16.0  # fp8 pre-scale (power of 2: keeps values out of subnormals)
ESC = 10.0 / (SCALE * SCALE)  # exp scale on device
EPS = 1e-12
WARM_MM = 6  # PE warm-up dummy matmuls issued during the DMA window

_CACHE = {}


def _build():
    from contextlib import ExitStack

    import concourse.bacc as bacc
    import concourse.tile as tile
    from concourse import masks, mybir

    f32 = mybir.dt.float32
    f16 = mybir.dt.float16
    f8 = mybir.dt.float8e4
    AF = mybir.ActivationFunctionType
    AX = mybir.AxisListType
    DR = mybir.MatmulPerfMode.DoubleRow

    nc = bacc.Bacc(None)
    # ft split by m-halves (columns), rh split by nh-halves
    fta_d = nc.dram_tensor("fta", [P, KB, 2, BPC // 2], f8, kind="ExternalInput")
    ftb_d = nc.dram_tensor("ftb", [P, KB, 2, BPC // 2], f8, kind="ExternalInput")
    rha_d = nc.dram_tensor("rha", [P, KB, 2, NH], f8, kind="ExternalInput")
    rhb_d = nc.dram_tensor("rhb", [P, KB, 2, NH], f8, kind="ExternalInput")
    res_out = nc.dram_tensor("res", [16, P], f32, kind="ExternalOutput")

    with tile.TileContext(nc) as tc, ExitStack() as ctx:
        pers = ctx.enter_context(tc.tile_pool(name="pers", bufs=1))
        scrp = ctx.enter_context(tc.tile_pool(name="scrp", bufs=3))
        pp = ctx.enter_context(tc.tile_pool(name="pp", bufs=3, space="PSUM"))
        pw = ctx.enter_context(tc.tile_pool(name="pw", bufs=1, space="PSUM"))
        pr = ctx.enter_context(tc.tile_pool(name="pr", bufs=1, space="PSUM"))

        fts = [pers.tile([P, KB, 2, BPC // 2], f8, name=f"ft{i}") for i in range(2)]
        rhs = [pers.tile([P, KB, 2, NH], f8, name=f"rh{i}") for i in range(2)]
        ses = pers.tile([P, 16], f32)
        ident = pers.tile([P, P], f32)
        wb1 = pers.tile([P, P], mybir.dt.bfloat16)
        wb2 = pers.tile([P, NH], mybir.dt.bfloat16)

        # warm fodder + exp-table warm tile, produced by vector immediately
        warm = pers.tile([1, 1], f32)
        nc.vector.memset(wb1[:], 0.0)
        nc.vector.memset(wb2[:], 0.0)
        nc.vector.memset(warm[:], 0.0)

        # input DMAs: first-needed first, spread across the three queues
        nc.sync.dma_start(out=rhs[0][:], in_=rha_d[:, :, :, :])
        nc.scalar.dma_start(out=fts[0][:], in_=fta_d[:, :, :, :])
        nc.sync.dma_start(out=rhs[1][:], in_=rhb_d[:, :, :, :])
        nc.gpsimd.dma_start(out=fts[1][:], in_=ftb_d[:, :, :, :])

        # load the Exp table while DMAs stream (scalar queue, after its
        # descriptor generation)
        nc.scalar.activation(out=warm[:], in_=warm[:], func=AF.Exp)

        masks.make_identity(nc, ident[:])

        # PE warm-up: keep the PE busy during the DMA window so HAM has
        # ramped the clock by the time the real matmuls arrive
        wps = pw.tile([P, NH], f32, tag="wps")
        for _ in range(WARM_MM):
            nc.tensor.matmul(
                wps[:], lhsT=wb1[:], rhs=wb2[:], start=True, stop=True
            )

        # m = 0..7: comp logits for own batch rows; m = 8: dis rows
        # (own classes are columns 0..127 of the rolled protoT)
        for m in range(MT + 1):
            pc = pp.tile([P, CPAD], f32, tag="pc", name=f"pc{m}")
            for nh in range(2):
                for kb in range(KB):
                    if m < MT:
                        lh = fts[m // 4][:, kb, :, (m % 4) * P : (m % 4 + 1) * P]
                    else:
                        lh = rhs[0][:, kb, :, 0:P]
                    nc.tensor.matmul(
                        pc[:, nh * NH : (nh + 1) * NH],
                        lhsT=lh,
                        rhs=rhs[nh][:, kb, :, :],
                        start=(kb == 0),
                        stop=(kb == KB - 1),
                        perf_mode=DR,
                    )
            # dis tile (m=8) keeps f32: its row-sum carries the e^10 diagonal
            # that the host subtracts exactly; f16 would round it by ~5e-4.
            if m < MT:
                e = scrp.tile([P, CPAD], f16, tag="e", name=f"e{m}")
            else:
                e = scrp.tile([P, CPAD], f32, tag="ed", name="ed")
            nc.scalar.activation(out=e[:], in_=pc[:], func=AF.Exp, scale=ESC)
            nc.vector.reduce_sum(out=ses[:, m : m + 1], in_=e[:], axis=AX.X)

        # transpose row-sums [128, 16] -> [16, 128] so the output DMA is
        # 16 big descriptors instead of 128 tiny ones
        pt = pr.tile([16, P], f32, tag="pt")
        nc.tensor.transpose(pt[:], ses[:], ident[:])
        sb = pers.tile([16, P], f32)
        nc.vector.tensor_copy(out=sb[:], in_=pt[:])
        nc.sync.dma_start(out=res_out[:, :], in_=sb[:])
    nc.finalize()
    return nc


def _get():
    if "nc" not in _CACHE:
        _CACHE["nc"] = _build()
    return _CACHE["nc"]


def _dr_layout(a):
    # [512, N] -> [128, 2, 2, N] with contraction row = kb*256 + j*128 + p
    n = a.shape[1]
    return np.ascontiguousarray(a.reshape(KB, 2, P, n).transpose(2, 0, 1, 3))


def kernel(features, prototypes, labels):
    import ml_dtypes

    from concourse.bass_utils import run_bass_kernel_spmd

    f32 = np.float32
    f8 = ml_dtypes.float8_e4m3
    features = np.ascontiguousarray(features, dtype=f32)
    prototypes = np.ascontiguousarray(prototypes, dtype=f32)
    labels = np.asarray(labels).astype(np.int64)

    # ---- exact EMA scan, vectorized across classes (order within a class
    # is batch order; classes are independent) ----
    order = np.argsort(labels, kind="stable")
    sf = features[order]
    sl = labels[order]
    counts = np.bincount(labels, minlength=C)
    starts = np.concatenate([[0], np.cumsum(counts)])[:-1]
    proto = prototypes.copy()
    for s in range(int(counts.max())):
        sel = counts > s
        idx = starts[sel] + s
        cls = sl[idx]
        upd = proto[cls] * f32(0.5) + sf[idx] * f32(0.5)
        n = np.sqrt(np.sum(upd * upd, axis=1, keepdims=True, dtype=f32))
        proto[cls] = upd / np.maximum(n, f32(EPS))

    pn = proto / np.maximum(
        np.sqrt(np.sum(proto * proto, axis=1, keepdims=True, dtype=f32)), f32(EPS)
    )

    # ---- quantize (scaled), pad classes to 1024, build device layouts ----
    pTq = np.zeros((D, CPAD), f8)
    pTq[:, :C] = (pn.T * f32(SCALE)).astype(f8)
    fTq = (features.T * f32(SCALE)).astype(f8)
    ft_dr = _dr_layout(fTq)  # [128, 2, 2, 8192]

    pq32 = pTq.astype(f32)
    rsq_scaled = np.sum(pq32 * pq32, axis=0, dtype=f32)  # [1024]

    # exact positive term in f64 (unquantized prototypes)
    cs = np.zeros((C, D), np.float64)
    np.add.at(cs, labels, features.astype(np.float64))
    possum = float(np.sum(cs * pn.astype(np.float64)))

    in_maps = []
    for c in range(NCORES):
        # roll so core c's own classes are columns 0..124
        rh = _dr_layout(np.roll(pTq, -c * CPC, axis=1))
        b0 = c * BPC
        in_maps.append(
            {
                "fta": np.ascontiguousarray(ft_dr[:, :, :, b0 : b0 + BPC // 2]),
                "ftb": np.ascontiguousarray(
                    ft_dr[:, :, :, b0 + BPC // 2 : b0 + BPC]
                ),
                "rha": np.ascontiguousarray(rh[:, :, :, :NH]),
                "rhb": np.ascontiguousarray(rh[:, :, :, NH:]),
            }
        )

    ncb = _get()
    res = run_bass_kernel_spmd(ncb, in_maps, list(range(NCORES))).results

    # ---- host combine (f64; ln of 8192 + 1000 row-sums) ----
    comp_total = 0.0
    dis_total = 0.0
    for c in range(NCORES):
        r = res[c]["res"].astype(np.float64)  # [16, 128]
        comp_total += np.sum(np.log(r[:MT, :] - NPAD))
        c0 = c * CPC
        diag = np.exp(rsq_scaled[c0 : c0 + CPC].astype(np.float64) * ESC)
        dis_total += np.sum(np.log(r[MT, :CPC] - diag - NPAD))

    mean_log_prob_pos = (10.0 * possum - comp_total) / B
    loss_comp = -mean_log_prob_pos
    loss_dis = dis_total / C - np.log(float(C - 1))
    return np.array(loss_comp + loss_dis, dtype=f32)


# revision 14
# speedup vs baseline: 1.0217x; 1.0217x over previous
"""CIDER criterion (DisLoss + CompLoss) on 8 Trainium2 NeuronCores.

Single launch per core. Host does index-driven data prep (the exact
per-sample EMA prototype scan, vectorized over classes), normalization,
fp8 quantization and layout; the device does the two large matmuls
(comp logits [1024 x 1000 x 512] per core, dis logits [128 x 1000 x 512])
plus the exp row-sums, which is >99.9% of the FLOPs.

Key design points:
- Operands pre-scaled by 16, quantized to fp8 e4m3 (TRN float8e4), in
  MatmulPerfMode.DoubleRow layout: SBUF tile [p, kb, j, col] contracts
  row kb*256 + j*128 + p. 0.5 cycles/output element = 2x bf16 rate.
- protoT columns are rolled per core so its own 125 dis classes always
  sit at columns 0..124 (comp row-sums are permutation invariant), so
  one static program serves all 8 cores with no per-core proto input.
- protoT and featT ship as ONE hbm tensor / ONE dma_start issued by the
  Sync engine: the other engines' sequencers release ~3us later than
  Sync, and HWDGE descriptor generation (~20ns/desc) plus doorbell
  latency made multi-queue splits slower, not faster (measured).
- Comp class columns split 512/488 so each matmul output stays inside
  one PSUM bank (matmul accumulation is bank-granular); the exp reads
  the contiguous [128, 1000] result in one pass.
- The dis row-sum goes through ACT accum_out (f32) instead of the DVE
  reduce chain: DVE at ~1.13us per [128,1000] reduce is the pipeline
  rate limiter, ACT has slack.
- Device ships raw exp row-sums [16, 128] (PE-transposed, 16 big
  descriptors); host applies ln, the exact e^10 diagonal correction
  (from the same fp8 values the PE sees), and the exact positive term
  sum_c <cs_c, p_c> in f64.
"""

import numpy as np

# ---- problem constants (hardcoded per the harness contract) ----
B, C, D = 8192, 1000, 512
NCORES = 8
CPC = C // NCORES  # 125 classes per core
BPC = B // NCORES  # 1024 batch rows per core
P = 128
MT = BPC // P  # 8 batch tiles per core
KB = 2  # two (128 x 2)-row DoubleRow contraction blocks
# packed free columns per (kb, j): 1024 feat | 1000 proto | 24 zero pad
# (power-of-two strides and 128-aligned weight offsets for the LDWEIGHTS ISA)
FREE = 2048
PO = BPC  # proto column base offset (1024)
SCALE = 16.0  # fp8 pre-scale (power of 2: keeps values out of subnormals)
ESC = 10.0 / (SCALE * SCALE)  # exp scale on device
EPS = 1e-12

_CACHE = {}


def _build():
    from contextlib import ExitStack

    import concourse.bacc as bacc
    import concourse.tile as tile
    from concourse import masks, mybir

    f32 = mybir.dt.float32
    f16 = mybir.dt.float16
    f8 = mybir.dt.float8e4
    AF = mybir.ActivationFunctionType
    AX = mybir.AxisListType
    DR = mybir.MatmulPerfMode.DoubleRow

    nc = bacc.Bacc(None)
    inp_d = nc.dram_tensor("inp", [P, KB, 2, FREE], f8, kind="ExternalInput")
    res_out = nc.dram_tensor("res", [16, P], f32, kind="ExternalOutput")

    # comp class-column split keeping each matmul output inside one PSUM bank
    nsplit = [(PO, PO + 512), (PO + 512, PO + C)]

    with tile.TileContext(nc) as tc, ExitStack() as ctx:
        pers = ctx.enter_context(tc.tile_pool(name="pers", bufs=1))
        scrp = ctx.enter_context(tc.tile_pool(name="scrp", bufs=3))
        pp = ctx.enter_context(tc.tile_pool(name="pp", bufs=4, space="PSUM"))

        it = pers.tile([P, KB, 2, FREE], f8)
        ses = pers.tile([P, 16], f32)
        ident = pers.tile([P, P], f32)

        # the one input DMA: 128 contiguous 8KB descriptors on the Sync
        # queue (the only engine whose sequencer is released early)
        nc.sync.dma_start(out=it[:], in_=inp_d[:, :, :, :])

        # load the Exp activation table while the DMA streams
        warm = pers.tile([1, 1], f32)
        nc.vector.memset(warm[:], 0.0)
        nc.scalar.activation(out=warm[:], in_=warm[:], func=AF.Exp)

        masks.make_identity(nc, ident[:])

        # m = 0..7: comp logits for own batch rows; m = 8: dis rows
        # (own classes are columns 0..127 of the rolled protoT)
        for m in range(MT + 1):
            pc = pp.tile([P, 1024], f32, tag="pc", name=f"pc{m}")
            for ni, (n0, n1) in enumerate(nsplit):
                for kb in range(KB):
                    if m < MT:
                        lh = it[:, kb, :, m * P : (m + 1) * P]
                    else:
                        lh = it[:, kb, :, PO : PO + P]
                    nc.tensor.matmul(
                        pc[:, 512 * ni : 512 * ni + (n1 - n0)],
                        lhsT=lh,
                        rhs=it[:, kb, :, n0:n1],
                        start=(kb == 0),
                        stop=(kb == KB - 1),
                        perf_mode=DR,
                    )
            # the dis tile (m=8) keeps f32 (its row-sum carries the e^10
            # diagonal the host subtracts exactly) and sums via ACT
            # accum_out, off the DVE chain that limits the pipeline rate.
            if m < MT:
                e = scrp.tile([P, 1024], f16, tag="e", name=f"e{m}")
                nc.scalar.activation(
                    out=e[:, 0:C], in_=pc[:, 0:C], func=AF.Exp, scale=ESC
                )
                nc.vector.reduce_sum(
                    out=ses[:, m : m + 1], in_=e[:, 0:C], axis=AX.X
                )
            else:
                e = scrp.tile([P, 1024], f32, tag="ed", name="ed")
                nc.scalar.activation(
                    out=e[:, 0:C],
                    in_=pc[:, 0:C],
                    func=AF.Exp,
                    scale=ESC,
                    accum_out=ses[:, m : m + 1],
                )

        # transpose row-sums [128, 16] -> [16, 128] so the output DMA is
        # 16 big descriptors instead of 128 tiny ones
        pt = pp.tile([P, 1024], f32, tag="pc", name="pt")
        nc.tensor.transpose(pt[:16, 0:P], ses[:], ident[:])
        sb = pers.tile([16, P], f32)
        nc.vector.tensor_copy(out=sb[:], in_=pt[:16, 0:P])
        nc.sync.dma_start(out=res_out[:, :], in_=sb[:])
    nc.finalize()
    return nc


def _get():
    if "nc" not in _CACHE:
        _CACHE["nc"] = _build()
    return _CACHE["nc"]


def _dr_layout(a):
    # [512, N] -> [128, 2, 2, N] with contraction row = kb*256 + j*128 + p
    n = a.shape[1]
    return np.ascontiguousarray(a.reshape(KB, 2, P, n).transpose(2, 0, 1, 3))


def kernel(features, prototypes, labels):
    import ml_dtypes

    from concourse.bass_utils import run_bass_kernel_spmd

    f32 = np.float32
    f8 = ml_dtypes.float8_e4m3
    features = np.ascontiguousarray(features, dtype=f32)
    prototypes = np.ascontiguousarray(prototypes, dtype=f32)
    labels = np.asarray(labels).astype(np.int64)

    # ---- exact EMA scan, vectorized across classes (order within a class
    # is batch order; classes are independent) ----
    order = np.argsort(labels, kind="stable")
    sf = features[order]
    sl = labels[order]
    counts = np.bincount(labels, minlength=C)
    starts = np.concatenate([[0], np.cumsum(counts)])[:-1]
    proto = prototypes.copy()
    for s in range(int(counts.max())):
        sel = counts > s
        idx = starts[sel] + s
        cls = sl[idx]
        upd = proto[cls] * f32(0.5) + sf[idx] * f32(0.5)
        n = np.sqrt(np.sum(upd * upd, axis=1, keepdims=True, dtype=f32))
        proto[cls] = upd / np.maximum(n, f32(EPS))

    pn = proto / np.maximum(
        np.sqrt(np.sum(proto * proto, axis=1, keepdims=True, dtype=f32)), f32(EPS)
    )

    # ---- quantize (scaled), build device layouts ----
    pTq = (pn.T * f32(SCALE)).astype(f8)  # [512, 1000]
    fTq = (features.T * f32(SCALE)).astype(f8)
    ft_dr = _dr_layout(fTq)  # [128, 2, 2, 8192]

    pq32 = pTq.astype(f32)
    rsq_scaled = np.sum(pq32 * pq32, axis=0, dtype=f32)  # [1000]

    # exact positive term in f64 (unquantized prototypes)
    cs = np.zeros((C, D), np.float64)
    np.add.at(cs, labels, features.astype(np.float64))
    possum = float(np.sum(cs * pn.astype(np.float64)))

    in_maps = []
    for c in range(NCORES):
        # roll so core c's own classes are columns 0..124, then pack
        # [proto | feat] into one [128, 2, 2, 2024] tensor
        rh = _dr_layout(np.roll(pTq, -c * CPC, axis=1))
        b0 = c * BPC
        combined = np.zeros((P, KB, 2, FREE), f8)
        combined[:, :, :, :BPC] = ft_dr[:, :, :, b0 : b0 + BPC]
        combined[:, :, :, PO : PO + C] = rh
        in_maps.append({"inp": combined})

    ncb = _get()
    res = run_bass_kernel_spmd(ncb, in_maps, list(range(NCORES))).results

    # ---- host combine (f64; ln of 8192 + 1000 row-sums) ----
    comp_total = 0.0
    dis_total = 0.0
    for c in range(NCORES):
        r = res[c]["res"].astype(np.float64)  # [16, 128]
        comp_total += np.sum(np.log(r[:MT, :]))
        c0 = c * CPC
        diag = np.exp(rsq_scaled[c0 : c0 + CPC].astype(np.float64) * ESC)
        dis_total += np.sum(np.log(r[MT, :CPC] - diag))

    mean_log_prob_pos = (10.0 * possum - comp_total) / B
    loss_comp = -mean_log_prob_pos
    loss_dis = dis_total / C - np.log(float(C - 1))
    return np.array(loss_comp + loss_dis, dtype=f32)
